# revision 1
# baseline (speedup 1.0000x reference)
"""CliffordBatchNormMV Trainium2 kernel.

Math (per grade g, block nb, batch token b):
  sumsq[b,nb,g] = sum_{c in grade g} x[b,nb,c]^2
  n = sqrt(sumsq + EPS)                       # grade norm
  mean/var over b (biased)                    # batch stats per (g, nb)
  inv = 1/sqrt(var + EPS)
  out[c] = x[c] * (A[g,nb] + C[g,nb] / n),  A = gs*gamma*inv,
                                            C = gs*(beta - gamma*inv*mean)

Distribution: shard the 64 nb-blocks across 8 cores (8 each) -> batch stats
are fully core-local, no collectives.

Per-core layout: host pre-transposes the shard to [8, 4096, 256] (nb-major).
Each nb-group (4096 tokens) streams through in 512-token chunks (4 tiles of
[128 tok, 256 mv]; token t of group = p*32 + q for partition p, slot q;
2 MiB DMAs cover 4 chunks):
  pass 1 per chunk: PE-transpose x -> PSUM [mv, tok], ACT Square -> x2T
    (f32r), f32r matmul against the 0/1 grade-membership matrix G ->
    sumsq [9, 512] PSUM (and a second accumulating matmul pair into a
    persistent stats bank = sum over the batch of sumsq), ACT Sqrt
    (+accum_out = sum of norms) -> gnorm, DVE reciprocal_approx_fast ->
    rgnorm (kept for pass 2).
  stats per group: E[n] from the sqrt accums, E[n^2] from the stats bank;
    var = E[n^2] - mean^2 (+EPS folded into the Sqrt bias); inv via
    Sqrt + reciprocal_approx_fast; A = gg*inv, C = gb - A*mean.
  pass 2 per chunk: s = C*rgnorm + A (DVE tensor_scalar, f32r), f32r
    matmul with G9 [9, 256] expands s to [128 tok, 256 mv] PSUM, DVE
    multiplies in-place into the resident x tile, DMA out.

All engines stay below the ~187 us/core DMA floor (64 MiB I/O at
~360 GB/s); cost-model timeline estimate ~215 us/core.
"""

import os
import numpy as np

MV = 256
NG = 9
EPS = 1e-5
B = 4096
NB = 64
N_CORES = 8
NB_PER_CORE = NB // N_CORES      # 8 nb-groups per core
QS = 32                          # token slots per partition per group
NCHUNK = 8                       # chunks per group
TPC = 4                          # token-tiles per chunk (512 tokens)

_GRADES = np.array([bin(i).count("1") for i in range(MV)])

LAST_RESULTS = None
_CACHE = {}


def _build_program():
    import concourse.bacc as bacc
    import concourse.tile as tile
    from concourse import mybir

    f32 = mybir.dt.float32
    f32r = mybir.dt.float32r
    AF = mybir.ActivationFunctionType
    Alu = mybir.AluOpType

    G_full = np.zeros((MV, NG), dtype=np.float32)
    G_full[np.arange(MV), _GRADES] = 1.0
    ident_np = np.eye(128, dtype=np.float32)

    nc = bacc.Bacc()
    x_in = nc.dram_tensor("x", [NB_PER_CORE, B, MV], f32, kind="ExternalInput")
    gg_in = nc.dram_tensor("gg", [NG, NB_PER_CORE], f32, kind="ExternalInput")
    gb_in = nc.dram_tensor("gb", [NG, NB_PER_CORE], f32, kind="ExternalInput")
    out_d = nc.dram_tensor("out", [NB_PER_CORE, B, MV], f32, kind="ExternalOutput")

    G_lo_c = nc.inline_tensor(G_full[:128], name="Glo")
    G_hi_c = nc.inline_tensor(G_full[128:], name="Ghi")
    G9_c = nc.inline_tensor(np.ascontiguousarray(G_full.T), name="G9")
    I_c = nc.inline_tensor(ident_np, name="Ident")

    inv_B = 1.0 / B

    with tile.TileContext(nc) as tc:
        with (
            tc.tile_pool(name="const", bufs=1) as const,
            tc.tile_pool(name="xc", bufs=int(os.environ.get("K_XC", "7"))) as xcp,
            tc.tile_pool(name="work", bufs=int(os.environ.get("K_WORK", "3"))) as work,
            tc.tile_pool(name="grp", bufs=int(os.environ.get("K_GRP", "2"))) as grp,
            tc.tile_pool(name="statp", bufs=2) as statp,
            tc.tile_pool(name="ps_xt", bufs=int(os.environ.get("K_PSXT", "2")), space="PSUM") as ps_xt,
            tc.tile_pool(name="ps_s", bufs=int(os.environ.get("K_PSS", "1")), space="PSUM") as ps_s,
            tc.tile_pool(name="ps_st", bufs=int(os.environ.get("K_PSST", "1")), space="PSUM") as ps_st,
            tc.tile_pool(name="ps_a", bufs=int(os.environ.get("K_PSA", "2")), space="PSUM") as ps_a,
        ):
            Glo = const.tile([128, NG], f32r)
            nc.gpsimd.dma_start(out=Glo, in_=G_lo_c[:, :])
            Ghi = const.tile([128, NG], f32r)
            nc.gpsimd.dma_start(out=Ghi, in_=G_hi_c[:, :])
            G9 = const.tile([NG, MV], f32r)
            nc.gpsimd.dma_start(out=G9, in_=G9_c[:, :])
            ident = const.tile([128, 128], f32)
            nc.sync.dma_start(out=ident, in_=I_c[:, :])
            gg = const.tile([NG, NB_PER_CORE], f32)
            nc.sync.dma_start(out=gg, in_=gg_in[:, :])
            gb = const.tile([NG, NB_PER_CORE], f32)
            nc.sync.dma_start(out=gb, in_=gb_in[:, :])
            eps9 = const.tile([NG, 1], f32)
            nc.vector.memset(eps9, EPS)
            eps2_9 = const.tile([NG, 1], f32)
            nc.vector.memset(eps2_9, 2.0 * EPS)

            for g in range(int(os.environ.get("K_NGROUPS", str(NB_PER_CORE)))):
                xv = x_in[g].rearrange("(p q) c -> p q c", q=QS)
                ov = out_d[g].rearrange("(p q) c -> p q c", q=QS)

                rg = grp.tile([NG, NCHUNK, 512], f32, tag="rg")
                gsum = grp.tile([NG, NCHUNK], f32, tag="gsum")
                st_ps = ps_st.tile([NG, 512], f32, tag="stps")

                xps = []
                CPT = int(os.environ.get("K_CPT", "4"))
                # ---- pass 1: norms + stat accumulators ----
                for ch in range(NCHUNK):
                    if ch % CPT == 0:
                        xp = xcp.tile([128, CPT * TPC, MV], f32, tag="xc")
                        xps.append(xp)
                        nc.sync.dma_start(
                            out=xp, in_=xv[:, ch * TPC:(ch + CPT) * TPC, :]
                        )
                    xc = xps[ch // CPT][:, (ch % CPT) * TPC:(ch % CPT + 1) * TPC, :]
                    xt_lo = ps_xt.tile([128, 512], f32, tag="xtlo")
                    xt_hi = ps_xt.tile([128, 512], f32, tag="xthi")
                    for k in range(TPC):
                        nc.tensor.transpose(
                            xt_lo[:, k * 128:(k + 1) * 128], xc[:, k, 0:128], ident
                        )
                        nc.tensor.transpose(
                            xt_hi[:, k * 128:(k + 1) * 128], xc[:, k, 128:256], ident
                        )
                    x2lo = work.tile([128, 512], f32r, tag="x2lo")
                    nc.scalar.activation(out=x2lo, in_=xt_lo, func=AF.Square)
                    x2hi = work.tile([128, 512], f32r, tag="x2hi")
                    nc.scalar.activation(out=x2hi, in_=xt_hi, func=AF.Square)

                    ps = ps_s.tile([NG, 512], f32, tag="ps")
                    nc.tensor.matmul(ps, Glo, x2lo, start=True, stop=False)
                    nc.tensor.matmul(ps, Ghi, x2hi, start=False, stop=True)
                    # accumulate sum over all tokens of sumsq into stats bank
                    nc.tensor.matmul(st_ps, Glo, x2lo,
                                     start=(ch == 0), stop=False)
                    nc.tensor.matmul(st_ps, Ghi, x2hi,
                                     start=False, stop=(ch == NCHUNK - 1))

                    if ch % 2 == 0:
                        gn2 = work.tile([NG, 2, 512], f32, tag="gn2")
                    nc.scalar.activation(
                        out=gn2[:, ch % 2, :], in_=ps, func=AF.Sqrt,
                        bias=eps9[:, 0:1], accum_out=gsum[:, ch:ch + 1],
                    )
                    if ch % 2 == 1:
                        nc.vector.reciprocal_approx_fast(
                            out=rg[:, ch - 1:ch + 1, :].rearrange("p a b -> p (a b)"),
                            in_=gn2.rearrange("p a b -> p (a b)"),
                        )

                # ---- batch stats -> A, C ----
                en2 = statp.tile([NG, 1], f32, tag="en2")
                nc.vector.tensor_reduce(
                    out=en2, in_=st_ps, axis=mybir.AxisListType.X, op=Alu.add
                )
                mn = statp.tile([NG, 1], f32, tag="mn")   # -mean
                nc.vector.tensor_reduce(
                    out=mn, in_=gsum, axis=mybir.AxisListType.X, op=Alu.add
                )
                nc.vector.tensor_scalar(
                    out=mn, in0=mn, scalar1=-inv_B, scalar2=None, op0=Alu.mult
                )
                m2 = statp.tile([NG, 1], f32, tag="m2")
                nc.vector.tensor_mul(m2, mn, mn)
                # var + EPS = en2/B + 2*EPS - mean^2   (en2 lacks the +EPS)
                var = statp.tile([NG, 1], f32, tag="var")
                nc.vector.scalar_tensor_tensor(
                    out=var, in0=en2, scalar=inv_B, in1=m2,
                    op0=Alu.mult, op1=Alu.subtract,
                )
                sd = statp.tile([NG, 1], f32, tag="sd")
                nc.scalar.activation(
                    out=sd, in_=var, func=AF.Sqrt, bias=eps2_9[:, 0:1]
                )
                inv = statp.tile([NG, 1], f32, tag="inv")
                nc.vector.reciprocal_approx_fast(out=inv, in_=sd)
                A = statp.tile([NG, 1], f32, tag="A")
                nc.vector.tensor_mul(A, gg[:, g:g + 1], inv)
                C = statp.tile([NG, 1], f32, tag="C")
                nc.vector.scalar_tensor_tensor(
                    out=C, in0=A, scalar=mn, in1=gb[:, g:g + 1],
                    op0=Alu.mult, op1=Alu.add,
                )

                # ---- pass 2: apply ----
                for ch in range(NCHUNK):
                    xc = xps[ch // CPT][:, (ch % CPT) * TPC:(ch % CPT + 1) * TPC, :]
                    if ch % 2 == 0:
                        s2 = work.tile([NG, 2, 512], f32r, tag="s2")
                        s_eng = nc.gpsimd if os.environ.get("K_SPOOL", "0") == "1" else nc.vector
                        s_eng.tensor_scalar(
                            out=s2.rearrange("p a b -> p (a b)"),
                            in0=rg[:, ch:ch + 2, :].rearrange("p a b -> p (a b)"),
                            scalar1=C, scalar2=A,
                            op0=Alu.mult, op1=Alu.add,
                        )
                    s = s2[:, ch % 2, :]
                    for half in range(2):
                        pa = ps_a.tile([128, 2, MV], f32, tag="pa")
                        for j in range(2):
                            k = half * 2 + j
                            nc.tensor.matmul(
                                pa[:, j, :], s[:, k * 128:(k + 1) * 128], G9,
                                start=True, stop=True,
                            )
                        q0 = half * 2
                        nc.vector.tensor_mul(
                            xc[:, q0:q0 + 2, :], xc[:, q0:q0 + 2, :], pa
                        )
                    if g == NB_PER_CORE - 1:
                        nc.sync.dma_start(
                            out=ov[:, ch * TPC:(ch + 1) * TPC, :], in_=xc
                        )
                    elif ch % CPT == CPT - 1:
                        nc.sync.dma_start(
                            out=ov[:, (ch - CPT + 1) * TPC:(ch + 1) * TPC, :],
                            in_=xps[ch // CPT],
                        )

    nc.compile()
    return nc


def kernel(x, gamma, beta, grade_scale):
    global LAST_RESULTS
    from concourse.bass_utils import run_bass_kernel_spmd

    if "nc" not in _CACHE:
        _CACHE["nc"] = _build_program()
    nc = _CACHE["nc"]

    x = np.asarray(x)
    assert x.shape == (B, NB, MV) and x.dtype == np.float32, (x.shape, x.dtype)
    gamma = np.asarray(gamma, dtype=np.float32)
    beta = np.asarray(beta, dtype=np.float32)
    grade_scale = np.asarray(grade_scale, dtype=np.float32)

    gg = grade_scale[:, None] * gamma          # [9, 64]
    gb = grade_scale[:, None] * beta           # [9, 64]

    x_t = x.transpose(1, 0, 2)                 # [64, 4096, 256] (view)
    in_maps = []
    for i in range(N_CORES):
        sl = slice(i * NB_PER_CORE, (i + 1) * NB_PER_CORE)
        in_maps.append({
            "x": np.ascontiguousarray(x_t[sl]),
            "gg": np.ascontiguousarray(gg[:, sl]),
            "gb": np.ascontiguousarray(gb[:, sl]),
        })

    want_trace = bool(int(os.environ.get("KERNEL_TRACE", "0") or "0"))
    if want_trace:
        # tracing under axon needs the NTFF hook; fall back cleanly if absent
        try:
            from antenv.axon_hooks import get_axon_ntff_profile_hook
            want_trace = get_axon_ntff_profile_hook() is not None
        except Exception:
            want_trace = False
    # retry: the axon relay occasionally returns a transient
    # NRT_EXEC_UNIT_UNRECOVERABLE; a rerun succeeds
    last_exc = None
    for _attempt in range(3):
        try:
            res = run_bass_kernel_spmd(
                nc, in_maps, core_ids=list(range(N_CORES)), trace=want_trace,
            )
            break
        except Exception as e:
            last_exc = e
            import time as _time
            _time.sleep(2.0)
    else:
        raise last_exc
    LAST_RESULTS = res

    out_t = np.concatenate([res.results[i]["out"] for i in range(N_CORES)], axis=0)
    out = np.ascontiguousarray(out_t.transpose(1, 0, 2)).astype(np.float32, copy=False)
    return out



# revision 21
# speedup vs baseline: 1.1172x; 1.1172x over previous
"""CliffordBatchNormMV Trainium2 kernel (bf16 I/O, mv-major layout, tall
per-token norm math).

Math (per grade g, block nb, batch token b):
  sumsq[g,b] = sum_{c in grade g} x[c,b]^2
  n = sqrt(sumsq + EPS)                      # grade norm
  mean/var over b (biased)                   # batch stats per (g, nb)
  inv = 1/sqrt(var + EPS)
  out[c,b] = x[c,b] * s[g(c),b],  s = A + C/n,  A = gs*gamma*inv,
                                  C = gs*(beta - gamma*inv*mean)

Distribution: shard the 64 nb-blocks across 8 cores (8 each) -> batch stats
are fully core-local, no collectives.

I/O in bf16 with the host pre-transposing each nb-group to mv-major
[256, 4096] (and inverting afterwards): halves HBM traffic vs f32 to a
~93 us/core DMA floor (32 MiB at 360 B/ns).

The per-token norm math runs in a TALL layout [128 tok, 4q, 10 grades]
(tokens on partitions) so sqrt/reciprocal cost free-size 40 instead of
512, and all batch reductions become tiny PE matmuls:
  pass 1 per chunk (512 tok = 4 q-slices of 128):
    x2 = x*x bf16 (ACT Square / gpsimd split); 8 small PE matmuls with the
    x2 q-slice as stationary and the 0/1 grade matrix [128,10] as moving
    -> ps_tall [128,4,10] f32 PSUM; ACT Sqrt(+eps) -> gn_tall f32r; PE
    ones-matmul accumulates sum(n) and a gn^T@gn Gram matmul accumulates
    sum(n^2) per grade across the whole group (PSUM accumulation); DVE
    reciprocal -> rn_tall (col 9 memset to 1.0: the expansion ones-row);
    4 PE permutation-transposes -> rnT [10,512] f32r PSUM; ACT Copy ->
    gradewise rn (kept for pass 2).
  stats per group: mean from the ones-matmul bank, E[n^2] from the Gram
  diagonal; inv via Sqrt + reciprocal; A, C [9,1] f32 exact. E_aug
  [10,256] f32r: rows 0..8 = C[g]*G9, row 9 = A[g(c)] (tiny PE matmul +
  ACT copy + row DMA).
  pass 2 per chunk: f32r expansion matmul pair E_aug^T @ rn_aug ->
  s = A + C*rn [128,2,512] f32 PSUM (exact f32 -> no cancellation error);
  DVE multiplies in-place into the resident x tile; DMA out.

Engine steady-state per chunk ~1.30-1.48 us vs the 1.456 us DMA floor;
single activation table set (sqrt/square/copy/identity) -> no reloads.
"""

import os
import numpy as np

MV = 256
NG = 9
NGP = 10                         # padded grade cols (col 9 = ones trick)
EPS = 1e-5
B = 4096
NB = 64
N_CORES = 8
NB_PER_CORE = NB // N_CORES      # 8 nb-groups per core
NCHUNK = 8                       # 512-token chunks per group
TOK = 512                        # tokens per chunk
NQ = 4                           # 128-token q-slices per chunk

_GRADES = np.array([bin(i).count("1") for i in range(MV)])

LAST_RESULTS = None
_CACHE = {}


def _build_program():
    import concourse.bacc as bacc
    import concourse.tile as tile
    from concourse import mybir

    f32 = mybir.dt.float32
    f32r = mybir.dt.float32r
    bf16 = mybir.dt.bfloat16
    AF = mybir.ActivationFunctionType
    Alu = mybir.AluOpType

    Gp = np.zeros((MV, NGP), dtype=np.float32)
    Gp[np.arange(MV), _GRADES] = 1.0          # 10th column stays zero
    G9 = np.ascontiguousarray(Gp[:, :NG].T)   # [9, 256]

    CPD = int(os.environ.get("K_CPD", "4"))       # chunks per DMA tile
    QA = int(os.environ.get("K_QA", "192"))       # x2 cols on ACT
    QD = int(os.environ.get("K_QD", "0"))         # x2 cols on DVE (tail)

    nc = bacc.Bacc()
    x_in = nc.dram_tensor("x", [NB_PER_CORE, MV, B], bf16, kind="ExternalInput")
    gg_in = nc.dram_tensor("gg", [NG, NB_PER_CORE], f32, kind="ExternalInput")
    gb_in = nc.dram_tensor("gb", [NG, NB_PER_CORE], f32, kind="ExternalInput")
    out_d = nc.dram_tensor("out", [NB_PER_CORE, MV, B], bf16, kind="ExternalOutput")

    G_lo_c = nc.inline_tensor(Gp[:128], name="Glo")
    G_hi_c = nc.inline_tensor(Gp[128:], name="Ghi")
    G9_c = nc.inline_tensor(G9, name="G9")
    I128_c = nc.inline_tensor(np.eye(128, dtype=np.float32), name="I128")
    I10_c = nc.inline_tensor(np.eye(NGP, dtype=np.float32), name="I10")
    ones_c = nc.inline_tensor(np.ones((128, 1), dtype=np.float32), name="ones")

    inv_B = 1.0 / B

    with tile.TileContext(nc) as tc:
        with (
            tc.tile_pool(name="const", bufs=1) as const,
            tc.tile_pool(name="xc", bufs=int(os.environ.get("K_XC", "6"))) as xcp,
            tc.tile_pool(name="x2p", bufs=int(os.environ.get("K_X2", "4"))) as x2p,
            tc.tile_pool(name="gnp", bufs=int(os.environ.get("K_GN", "3"))) as gnp,
            tc.tile_pool(name="grp", bufs=2) as grp,
            tc.tile_pool(name="statp", bufs=2) as statp,
            tc.tile_pool(name="ps_t", bufs=int(os.environ.get("K_PST", "1")), space="PSUM") as ps_t,
            tc.tile_pool(name="ps_r", bufs=int(os.environ.get("K_PSR", "1")), space="PSUM") as ps_r,
            tc.tile_pool(name="ps_g", bufs=1, space="PSUM") as ps_g,
            tc.tile_pool(name="ps_n", bufs=1, space="PSUM") as ps_n,
            tc.tile_pool(name="ps_x", bufs=int(os.environ.get("K_PSX", "2")), space="PSUM") as ps_x,
        ):
            Gmv_lo = const.tile([128, NGP], bf16)
            nc.gpsimd.dma_start(out=Gmv_lo, in_=G_lo_c[:, :])
            Gmv_hi = const.tile([128, NGP], bf16)
            nc.gpsimd.dma_start(out=Gmv_hi, in_=G_hi_c[:, :])
            G9f = const.tile([NG, MV], f32)
            nc.sync.dma_start(out=G9f, in_=G9_c[:, :])
            G9r = const.tile([NG, MV], f32r)
            nc.gpsimd.dma_start(out=G9r, in_=G9_c[:, :])
            I128 = const.tile([128, 128], f32r)
            nc.gpsimd.dma_start(out=I128, in_=I128_c[:, :])
            I10m = const.tile([NGP, NGP], f32)
            nc.sync.dma_start(out=I10m, in_=I10_c[:, :])
            gg = const.tile([NG, NB_PER_CORE], f32)
            nc.sync.dma_start(out=gg, in_=gg_in[:, :])
            gb = const.tile([NG, NB_PER_CORE], f32)
            nc.sync.dma_start(out=gb, in_=gb_in[:, :])
            ones_f = const.tile([128, 1], f32r)
            nc.gpsimd.dma_start(out=ones_f, in_=ones_c[:, :])
            ones41 = const.tile([128, NQ, 1], f32)
            nc.vector.memset(ones41, 1.0)
            eps_col = const.tile([128, 1], f32)
            nc.vector.memset(eps_col, EPS)
            epsg = const.tile([NGP, 1], f32)
            nc.vector.memset(epsg, EPS)

            for g in range(int(os.environ.get("K_NGROUPS", str(NB_PER_CORE)))):
                xv = x_in[g].rearrange("(h p) t -> h p t", h=2)   # [2,128,4096]
                ov = out_d[g].rearrange("(h p) t -> h p t", h=2)

                rgw = grp.tile([NGP, NCHUNK, TOK], f32r, tag="rgw")
                gram = ps_g.tile([NGP, NGP], f32, tag="gram")
                snb = ps_n.tile([1, TOK], f32, tag="snb")   # sum-n [1,0:10] + Aexp [0,256:512]

                xts = []
                # ---- pass 1: tall norms + PE-accumulated stats ----
                for ch in range(NCHUNK):
                    if ch % CPD == 0:
                        xt = xcp.tile([128, 2, CPD * TOK], bf16, tag="xc")
                        xts.append(xt)
                        sl = slice(ch * TOK, (ch + CPD) * TOK)
                        nc.sync.dma_start(out=xt[:, 0, :], in_=xv[0, :, sl])
                        nc.sync.dma_start(out=xt[:, 1, :], in_=xv[1, :, sl])
                    co = (ch % CPD) * TOK
                    xc = xts[ch // CPD][:, :, co:co + TOK]

                    x2 = x2p.tile([128, 2, TOK], bf16, tag="x2")
                    nc.scalar.activation(
                        out=x2[:, :, 0:QA], in_=xc[:, :, 0:QA], func=AF.Square
                    )
                    nc.gpsimd.tensor_mul(
                        x2[:, :, QA:TOK - QD], xc[:, :, QA:TOK - QD],
                        xc[:, :, QA:TOK - QD],
                    )
                    if QD:
                        nc.vector.tensor_mul(
                            x2[:, :, TOK - QD:TOK], xc[:, :, TOK - QD:TOK],
                            xc[:, :, TOK - QD:TOK],
                        )

                    pst = ps_t.tile([128, NQ, NGP], f32, tag="pst")
                    for q in range(NQ):
                        qs = slice(q * 128, (q + 1) * 128)
                        nc.tensor.matmul(pst[:, q, :], x2[:, 0, qs], Gmv_lo,
                                         start=True, stop=False)
                        nc.tensor.matmul(pst[:, q, :], x2[:, 1, qs], Gmv_hi,
                                         start=False, stop=True)

                    gn = gnp.tile([128, NQ, NGP], f32r, tag="gn")
                    nc.scalar.activation(
                        out=gn, in_=pst, func=AF.Sqrt, bias=eps_col[:, 0:1]
                    )
                    first = (ch == 0)
                    last = (ch == NCHUNK - 1)
                    for q in range(NQ):
                        nc.tensor.matmul(
                            snb[0:1, 0:NGP], ones_f, gn[:, q, :],
                            start=(first and q == 0), stop=(last and q == NQ - 1),
                        )
                        nc.tensor.matmul(
                            gram, gn[:, q, :], gn[:, q, :],
                            start=(first and q == 0), stop=(last and q == NQ - 1),
                        )

                    rn = gnp.tile([128, NQ, NGP], f32r, tag="rn")
                    with nc.allow_low_precision("f32r bits are f32"):
                        nc.vector.reciprocal(out=rn, in_=gn)
                    # col 9: sumsq is 0 there -> force rn to 1.0 so the
                    # expansion's A-row (ones moving row) works
                    nc.vector.tensor_scalar(
                        out=rn[:, :, NG:NGP], in0=ones41, scalar1=1.0,
                        scalar2=None, op0=Alu.mult,
                    )

                    rnT = ps_r.tile([NGP, TOK], f32r, tag="rnT")
                    for q in range(NQ):
                        nc.tensor.transpose(
                            rnT[:, q * 128:(q + 1) * 128], rn[:, q, :], I128
                        )
                    nc.scalar.activation(
                        out=rgw[:, ch, :], in_=rnT, func=AF.Copy
                    )

                # ---- batch stats -> A, C, E_aug ----
                # -mean row: -(sum n)/B, then a tiny DMA flips [1,10]->[10,1]
                mnr = statp.tile([1, NGP], f32, tag="mnr")
                nc.vector.tensor_scalar(
                    out=mnr, in0=snb[0:1, 0:NGP], scalar1=-inv_B, scalar2=None,
                    op0=Alu.mult,
                )
                mn = statp.tile([NGP, 1], f32, tag="mn")
                nc.sync.dma_start(out=mn, in_=mnr)
                # E[n^2]*B per grade = Gram diagonal
                gd = statp.tile([NGP, NGP], f32, tag="gd")
                nc.vector.tensor_mul(gd, gram, I10m)
                sv = statp.tile([NGP, 1], f32, tag="sv")
                nc.vector.tensor_reduce(
                    out=sv, in_=gd, axis=mybir.AxisListType.X, op=Alu.add
                )
                m2 = statp.tile([NGP, 1], f32, tag="m2")
                nc.gpsimd.tensor_mul(m2, mn, mn)
                # var + EPS = sv/B - mean^2 + EPS  (sv already includes +EPS)
                var = statp.tile([NGP, 1], f32, tag="var")
                nc.vector.scalar_tensor_tensor(
                    out=var, in0=sv, scalar=inv_B, in1=m2,
                    op0=Alu.mult, op1=Alu.subtract,
                )
                sd = statp.tile([NGP, 1], f32, tag="sd")
                nc.scalar.activation(
                    out=sd, in_=var, func=AF.Sqrt, bias=epsg[:, 0:1]
                )
                inv = statp.tile([NGP, 1], f32, tag="inv")
                nc.vector.reciprocal(out=inv, in_=sd)
                Ar = statp.tile([NG, 1], f32r, tag="Ar")
                nc.gpsimd.tensor_mul(Ar, gg[:, g:g + 1], inv[0:NG, :])
                Af = statp.tile([NG, 1], f32, tag="Af")
                nc.gpsimd.tensor_mul(Af, gg[:, g:g + 1], inv[0:NG, :])
                C = statp.tile([NG, 1], f32, tag="C")
                nc.vector.scalar_tensor_tensor(
                    out=C, in0=Af, scalar=mn[0:NG, :], in1=gb[:, g:g + 1],
                    op0=Alu.mult, op1=Alu.add,
                )
                # E_aug rows 0..8 = C[g]*G9 ; row 9 = A[g(c)]
                Eaug = statp.tile([NGP, MV], f32r, tag="Eaug")
                nc.vector.tensor_scalar(
                    out=Eaug[0:NG, :], in0=G9f, scalar1=C, scalar2=None,
                    op0=Alu.mult,
                )
                nc.tensor.matmul(snb[0:1, MV:2 * MV], Ar, G9r,
                                 start=True, stop=True)
                aex = statp.tile([1, MV], f32r, tag="aex")
                nc.scalar.activation(
                    out=aex, in_=snb[0:1, MV:2 * MV], func=AF.Copy
                )
                nc.sync.dma_start(out=Eaug[NG:NGP, :], in_=aex)

                # ---- pass 2: apply ----
                for ch in range(NCHUNK):
                    co = (ch % CPD) * TOK
                    xc = xts[ch // CPD][:, :, co:co + TOK]

                    sx = ps_x.tile([128, 2, TOK], f32, tag="sx")
                    nc.tensor.matmul(sx[:, 0, :], Eaug[:, 0:128], rgw[:, ch, :],
                                     start=True, stop=True)
                    nc.tensor.matmul(sx[:, 1, :], Eaug[:, 128:256], rgw[:, ch, :],
                                     start=True, stop=True)
                    nc.vector.tensor_mul(xc, xc, sx)

                    if ch % CPD == CPD - 1:
                        sl = slice((ch - CPD + 1) * TOK, (ch + 1) * TOK)
                        xt = xts[ch // CPD]
                        nc.sync.dma_start(out=ov[0, :, sl], in_=xt[:, 0, :])
                        nc.sync.dma_start(out=ov[1, :, sl], in_=xt[:, 1, :])

    nc.compile()
    return nc


def kernel(x, gamma, beta, grade_scale):
    global LAST_RESULTS
    import ml_dtypes
    from concourse.bass_utils import run_bass_kernel_spmd

    bf16 = ml_dtypes.bfloat16

    if "nc" not in _CACHE:
        _CACHE["nc"] = _build_program()
    nc = _CACHE["nc"]

    x = np.asarray(x)
    assert x.shape == (B, NB, MV) and x.dtype == np.float32, (x.shape, x.dtype)
    gamma = np.asarray(gamma, dtype=np.float32)
    beta = np.asarray(beta, dtype=np.float32)
    grade_scale = np.asarray(grade_scale, dtype=np.float32)

    gg = grade_scale[:, None] * gamma          # [9, 64]
    gb = grade_scale[:, None] * beta           # [9, 64]

    # mv-major per nb-block: [64, 256, 4096] bf16
    x_t = np.ascontiguousarray(x.transpose(1, 2, 0)).astype(bf16)
    in_maps = []
    for i in range(N_CORES):
        sl = slice(i * NB_PER_CORE, (i + 1) * NB_PER_CORE)
        in_maps.append({
            "x": np.ascontiguousarray(x_t[sl]),
            "gg": np.ascontiguousarray(gg[:, sl]),
            "gb": np.ascontiguousarray(gb[:, sl]),
        })

    want_trace = bool(int(os.environ.get("KERNEL_TRACE", "0") or "0"))
    if want_trace:
        # tracing under axon needs the NTFF hook; fall back cleanly if absent
        try:
            from antenv.axon_hooks import get_axon_ntff_profile_hook
            want_trace = get_axon_ntff_profile_hook() is not None
        except Exception:
            want_trace = False
    # retry: the axon relay occasionally returns a transient
    # NRT_EXEC_UNIT_UNRECOVERABLE; a rerun succeeds
    last_exc = None
    for _attempt in range(3):
        try:
            res = run_bass_kernel_spmd(
                nc, in_maps, core_ids=list(range(N_CORES)), trace=want_trace,
            )
            break
        except Exception as e:
            last_exc = e
            import time as _time
            _time.sleep(2.0)
    else:
        raise last_exc
    LAST_RESULTS = res

    out_t = np.concatenate(
        [np.asarray(res.results[i]["out"]) for i in range(N_CORES)], axis=0
    )                                          # [64, 256, 4096] bf16
    out = np.ascontiguousarray(
        out_t.transpose(2, 0, 1)
    ).astype(np.float32)                       # [4096, 64, 256]
    return out
